# revision 1
# baseline (speedup 1.0000x reference)
"""ALiBi causal attention block (QKV proj + attention + out proj) on 8 TRN2
NeuronCores, written in Bass/Tile.

Sharding: batch(2) x head-group(4) -> 8 cores; core c handles batch c//4 and
heads [4*(c%4), 4*(c%4)+4). QKV projection and attention are comm-free per
core. A per-head 8-core AllToAll (bf16 payload) redistributes attention
outputs from head-sharding to row-sharding; cross-batch shards carry
duplicate data that a per-core 0/1 blend masks out, keeping the SPMD graph
rank-agnostic. Each core then computes its 512 output rows against the full
Wo (bf16, half prefetched during attention) and writes rows
[512*(c%4), 512*(c%4+1)) of its batch.

Per-core pipeline (big matmuls in fp32r = full-rate fp32 PE mode):
  A: x -> PE-transpose -> xT tiles; qT/kT = W.T @ xT and v = xT.T @ W.
  B: per head, transposed layout [j, i]: scores on PE; ALiBi bias + causal
     mask in one fused DVE op; exp on ACT; the jt loop is software-pipelined
     so the denominator/AV matmuls of tile jt-1 are emitted after the scores
     matmul of tile jt -- the PE never head-of-line blocks on the exp.
  C: blended A2A shards (bf16) x Wo (bf16), 16 K-tiles into 8 PSUM banks
     per column half, + bo, DMA out.

Same algorithm as v2 (A2A-based output projection), plus:
- input/intermediate DMAs spread across sync/scalar/gpsimd queues
- attention outputs staged per (head, row-block) and shipped immediately
- blend of A2A shards emitted inside phase B so it overlaps attention
- diagonal bias+mask tiles built directly with affine_select
- single 3D-AP DMA for the per-head v load
"""

import math

import numpy as np

import concourse.bass as bass
import concourse.mybir as mybir
import concourse.tile as tile
from concourse import bacc
from concourse.bass_utils import run_bass_kernel_spmd
from concourse.masks import make_identity

F32 = mybir.dt.float32
F32R = mybir.dt.float32r
BF16 = mybir.dt.bfloat16
AL = mybir.AluOpType
AF = mybir.ActivationFunctionType

HIDDEN = 2048
NUM_HEADS = 16
HEAD = 128
SEQ = 2048
BATCH = 2
N_CORES = 8
HL = 4
QD = HL * HEAD
SCALE = 1.0 / math.sqrt(HEAD)
NEG = -1.0e6


def _slopes():
    if NUM_HEADS <= 8:
        return [1.0 / 2 ** k for k in range(NUM_HEADS)]
    return [1.0 / 2 ** (k / 2) for k in range(NUM_HEADS)]


def _r(ap):
    return ap.bitcast(F32R)


def build_nc(seq=SEQ, fp32r=True):
    E = HIDDEN
    ST = seq // 128
    S4 = seq // 512
    ET = E // 128
    RQ = seq // 4
    RT = RQ // 128
    cast = _r if fp32r else (lambda ap: ap)
    DTM = F32R if fp32r else F32

    nc = bacc.Bacc("TRN2", target_bir_lowering=False, debug=False,
                   num_devices=N_CORES)

    x_d = nc.dram_tensor("x", [seq, E], F32, kind="ExternalInput").ap()
    wq_d = nc.dram_tensor("wq", [E, QD], F32, kind="ExternalInput").ap()
    wk_d = nc.dram_tensor("wk", [E, QD], F32, kind="ExternalInput").ap()
    wv_d = nc.dram_tensor("wv", [E, QD], F32, kind="ExternalInput").ap()
    bq_d = nc.dram_tensor("bq", [QD], F32, kind="ExternalInput").ap()
    bk_d = nc.dram_tensor("bk", [QD], F32, kind="ExternalInput").ap()
    bv_d = nc.dram_tensor("bv", [QD], F32, kind="ExternalInput").ap()
    wo_d = nc.dram_tensor("wo", [E, E], BF16, kind="ExternalInput").ap()
    bo_d = nc.dram_tensor("bo", [E], F32, kind="ExternalInput").ap()
    bjv_d = nc.dram_tensor("bjv", [128, HL * S4 * ST], F32,
                           kind="ExternalInput").ap()
    bim_d = nc.dram_tensor("bim", [128, HL * 512], F32,
                           kind="ExternalInput").ap()
    zsel_d = nc.dram_tensor("zsel", [128, 2], F32, kind="ExternalInput").ap()
    out_d = nc.dram_tensor("out", [RQ, E], F32, kind="ExternalOutput").ap()

    with tile.TileContext(nc) as tc:
        with (
            tc.tile_pool(name="const", bufs=1) as cpool,
            tc.tile_pool(name="dram", bufs=1, space="DRAM") as dpool,
        ):
            ident = cpool.tile([128, 128], F32, name="ident")
            make_identity(nc, ident[:])
            ones_col = cpool.tile([128, 1], F32, name="ones_col")
            nc.gpsimd.memset(ones_col[:], 1.0)
            ones_col_r = cpool.tile([128, 1], F32R, name="ones_col_r")
            nc.vector.tensor_copy(ones_col_r[:], ones_col[:])
            ones_row = cpool.tile([1, 128], F32, name="ones_row")
            nc.gpsimd.memset(ones_row[:], 1.0)
            ones_row_r = cpool.tile([1, 128], F32R, name="ones_row_r")
            nc.vector.tensor_copy(ones_row_r[:], ones_row[:])
            zsel = cpool.tile([128, 2], F32, name="zsel")
            nc.sync.dma_start(zsel[:], zsel_d[:])

            qT_t = dpool.tile([QD, seq], F32, name="qT_i")
            kT_t = dpool.tile([QD, seq], F32, name="kT_i")
            v_t = dpool.tile([seq, QD], F32, name="v_i")
            a2a_in = [dpool.tile([N_CORES * 128, RQ], BF16, name=f"a2ai{h}")
                      for h in range(HL)]
            a2a_out = [dpool.tile([N_CORES * 128, RQ], BF16, name=f"a2ao{h}")
                       for h in range(HL)]

            # ---------------- Phase A ----------------
            with (
                tc.tile_pool(name="wp", bufs=1) as wp,
                tc.tile_pool(name="xp", bufs=4) as xp,
                tc.tile_pool(name="xtp", bufs=1) as xtp,
                tc.tile_pool(name="stgA", bufs=4) as stgA,
                tc.tile_pool(name="psA_t", bufs=3, space="PSUM") as psA_t,
                tc.tile_pool(name="psA_m", bufs=3, space="PSUM") as psA_m,
            ):
                xn0 = []
                for st in range(4):
                    t = xp.tile([128, E], F32, tag="xn", name="xn")
                    eng = nc.sync if st < 2 else nc.scalar
                    eng.dma_start(t[:], x_d[st * 128:(st + 1) * 128, :])
                    xn0.append(t)

                wt = {}
                wengine = {0: nc.scalar, 1: nc.scalar, 2: nc.gpsimd}
                for wi, wd in enumerate((wq_d, wk_d, wv_d)):
                    for et in range(ET):
                        t = wp.tile([128, QD], DTM, name=f"w{wi}_{et}")
                        wengine[wi].dma_start(
                            t[:], cast(wd[et * 128:(et + 1) * 128, :]))
                        wt[(wi, et)] = t
                bvec = {}
                for bi, bd in enumerate((bq_d, bk_d)):
                    for m in range(HL):
                        t = wp.tile([128, 1], F32, name=f"b{bi}_{m}")
                        nc.sync.dma_start(
                            t[:], bd[m * 128:(m + 1) * 128].rearrange(
                                "(p o) -> p o", o=1))
                        bvec[(bi, m)] = t
                bv_row = wp.tile([1, QD], F32, name="bv_row")
                nc.sync.dma_start(bv_row[:], bv_d.rearrange("(o q) -> o q", o=1))
                bv_bc = wp.tile([128, QD], F32, name="bv_bc")
                ps_bv = psA_m.tile([128, 512], F32, tag="mm", name="ps_bv")
                nc.tensor.matmul(ps_bv[:], ones_row[:], bv_row[:],
                                 start=True, stop=True)
                nc.scalar.copy(bv_bc[:], ps_bv[:])

                for s4 in range(S4):
                    if s4 == 0:
                        xn = xn0
                    else:
                        xn = []
                        for st in range(4):
                            t = xp.tile([128, E], F32, tag="xn", name="xn")
                            nc.sync.dma_start(
                                t[:], x_d[(s4 * 4 + st) * 128:
                                          (s4 * 4 + st + 1) * 128, :])
                            xn.append(t)
                    xT = [xtp.tile([128, 512], DTM, tag=f"xT{et}",
                                   name=f"xT{et}")
                          for et in range(ET)]
                    for et in range(ET):
                        for st in range(4):
                            pt = psA_t.tile([128, 128], F32, tag="tp",
                                            name="ps_tp")
                            nc.tensor.transpose(
                                pt[:], xn[st][:, et * 128:(et + 1) * 128],
                                ident[:])
                            nc.vector.tensor_copy(
                                xT[et][:, st * 128:(st + 1) * 128], pt[:])
                    for wi, dst in ((0, qT_t), (1, kT_t)):
                        for m in range(HL):
                            ps = psA_m.tile([128, 512], F32, tag="mm",
                                            name="ps_mm")
                            for et in range(ET):
                                nc.tensor.matmul(
                                    ps[:],
                                    wt[(wi, et)][:, m * 128:(m + 1) * 128],
                                    xT[et][:],
                                    start=(et == 0), stop=(et == ET - 1))
                            so = stgA.tile([128, 512], F32, tag="stgA",
                                           name="soA")
                            nc.scalar.activation(so[:], ps[:], AF.Identity,
                                                 bias=bvec[(wi, m)][:],
                                                 scale=1.0)
                            nc.sync.dma_start(
                                dst[m * 128:(m + 1) * 128,
                                    s4 * 512:(s4 + 1) * 512], so[:])
                    for st in range(4):
                        ps = psA_m.tile([128, 512], F32, tag="mm",
                                        name="ps_mv")
                        for et in range(ET):
                            nc.tensor.matmul(
                                ps[:],
                                xT[et][:, st * 128:(st + 1) * 128],
                                wt[(2, et)][:],
                                start=(et == 0), stop=(et == ET - 1))
                        so = stgA.tile([128, 512], F32, tag="stgA",
                                       name="soV")
                        nc.vector.scalar_tensor_tensor(
                            so[:], ps[:], 0.0, bv_bc[:], AL.bypass, AL.add)
                        nc.sync.dma_start(
                            v_t[(s4 * 4 + st) * 128:(s4 * 4 + st + 1) * 128, :],
                            so[:])

            # ---------------- Phase B + blends ----------------
            with (
                tc.tile_pool(name="hid", bufs=1) as hidp,
                tc.tile_pool(name="wo0", bufs=1) as wo0p,
            ):
              with (
                tc.tile_pool(name="ldp", bufs=4) as ldp,
                tc.tile_pool(name="blt", bufs=2) as blt,
                tc.tile_pool(name="att", bufs=3) as attp,
                tc.tile_pool(name="bc", bufs=1) as bcp,
                tc.tile_pool(name="kv", bufs=2) as kvp,
                tc.tile_pool(name="pp", bufs=6) as ppool,
                tc.tile_pool(name="stgB", bufs=3) as stgB,
                tc.tile_pool(name="psB_s", bufs=3, space="PSUM") as psB_s,
                tc.tile_pool(name="psB_o", bufs=2, space="PSUM") as psB_o,
                tc.tile_pool(name="psB_d", bufs=2, space="PSUM") as psB_d,
                tc.tile_pool(name="psB_b", bufs=1, space="PSUM") as psB_b,
            ):
                bjv = bcp.tile([128, HL * S4 * ST], F32, name="bjv")
                nc.sync.dma_start(bjv[:], bjv_d[:])
                bim = bcp.tile([128, HL * 512], F32, name="bim")
                nc.sync.dma_start(bim[:], bim_d[:])
                bimask = {}
                for hl in range(HL):
                    for r in range(4):
                        mt = bcp.tile([128, 512], F32, name=f"bms{hl}_{r}")
                        # where j<=i keep bim value, else ~NEG
                        nc.gpsimd.affine_select(
                            out=mt[:], in_=bim[:, hl * 512:(hl + 1) * 512],
                            compare_op=AL.is_ge, fill=NEG,
                            base=-r * 128, channel_multiplier=-1,
                            pattern=[[1, 512]])
                        bimask[(hl, r)] = mt

                wo0 = []
                for k in range(4 * HL):
                    hl_, src_ = k // 4, k % 4
                    eg = (src_ * 4 + hl_) * 128
                    t = wo0p.tile([128, 1024], BF16, name=f"wo0_{k}")
                    eng = nc.scalar if k % 2 == 0 else nc.sync
                    eng.dma_start(t[:], wo_d[eg:eg + 128, 0:1024])
                    wo0.append(t)

                hid = {}

                def emit_blend(hl):
                    for src_ in range(4):
                        k = hl * 4 + src_
                        la = ldp.tile([128, RQ], BF16, tag="la", name="la")
                        nc.sync.dma_start(
                            la[:],
                            a2a_out[hl][src_ * 128:(src_ + 1) * 128, :])
                        lb = ldp.tile([128, RQ], BF16, tag="lb", name="lb")
                        nc.sync.dma_start(
                            lb[:],
                            a2a_out[hl][(src_ + 4) * 128:(src_ + 5) * 128, :])
                        tmp = blt.tile([128, RQ], BF16, tag="tmp", name="tmp")
                        nc.vector.tensor_scalar(
                            tmp[:], lb[:], zsel[:, 1:2], None, AL.mult)
                        ht = hidp.tile([128, RQ], BF16, name=f"hid{k}")
                        nc.vector.scalar_tensor_tensor(
                            ht[:], la[:], zsel[:, 0:1], tmp[:],
                            AL.mult, AL.add)
                        hid[k] = ht

                for hl in range(HL):
                    kT = kvp.tile([128, seq], DTM, tag="kT", name="kTh")
                    qT = kvp.tile([128, seq], DTM, tag="qT", name="qTh")
                    if hl == 0:
                        for c4 in range(S4):
                            cs = slice(c4 * 512, (c4 + 1) * 512)
                            nc.sync.dma_start(
                                kT[:, cs],
                                cast(kT_t[hl * 128:(hl + 1) * 128, cs]))
                            nc.scalar.dma_start(
                                qT[:, cs],
                                cast(qT_t[hl * 128:(hl + 1) * 128, cs]))
                    else:
                        nc.sync.dma_start(
                            kT[:], cast(kT_t[hl * 128:(hl + 1) * 128, :]))
                        nc.scalar.dma_start(
                            qT[:], cast(qT_t[hl * 128:(hl + 1) * 128, :]))
                    vh = kvp.tile([128, seq], DTM, tag="vh", name="vh")
                    vsrc = v_t[:].rearrange(
                        "(jt p) (h d) -> p jt h d", p=128, d=128)[:, :, hl, :]
                    vdst = vh[:].rearrange("p (jt d) -> p jt d", d=128)
                    nc.sync.dma_start(vdst, cast(vsrc))
                    for im in range(S4):
                        ps_o = psB_o.tile([128, 512], F32, tag="o",
                                          name="ps_o")
                        ps_d = psB_d.tile([1, 512], F32, tag="d", name="ps_d")
                        njt = 4 * im + 4
                        prev = None

                        def consume(jt_, p_):
                            nc.tensor.matmul(ps_d[:], ones_col_r[:], p_[:],
                                             start=(jt_ == 0),
                                             stop=(jt_ == njt - 1))
                            nc.tensor.matmul(
                                ps_o[:],
                                vh[:, jt_ * 128:(jt_ + 1) * 128], p_[:],
                                start=(jt_ == 0), stop=(jt_ == njt - 1))

                        for jt in range(njt):
                            ps_s = psB_s.tile([128, 512], F32, tag="s",
                                              name="ps_s")
                            nc.tensor.matmul(
                                ps_s[:],
                                kT[:, jt * 128:(jt + 1) * 128],
                                qT[:, im * 512:(im + 1) * 512],
                                start=True, stop=True)
                            jv = bjv[:, (hl * S4 + im) * ST + jt:
                                     (hl * S4 + im) * ST + jt + 1]
                            r = jt - 4 * im
                            if r >= 0:
                                in1 = bimask[(hl, r)][:]
                            else:
                                in1 = bim[:, hl * 512:(hl + 1) * 512]
                            nc.vector.scalar_tensor_tensor(
                                ps_s[:], ps_s[:], jv, in1, AL.add, AL.add)
                            p = ppool.tile([128, 512], DTM, tag="p", name="p")
                            nc.scalar.activation(p[:], ps_s[:], AF.Exp,
                                                 scale=SCALE)
                            if prev is not None:
                                consume(*prev)
                            prev = (jt, p)
                        consume(*prev)
                        sd = stgB.tile([1, 512], F32R, tag="sd", name="sd")
                        nc.vector.tensor_copy(sd[:], ps_d[:])
                        ps_b = psB_b.tile([128, 512], F32, tag="b",
                                          name="ps_b")
                        nc.tensor.matmul(ps_b[:], ones_row_r[:], sd[:],
                                         start=True, stop=True)
                        sr = stgB.tile([128, 512], F32, tag="sr", name="sr")
                        nc.vector.reciprocal_approx_fast(sr[:], ps_b[:])
                        ao = attp.tile([128, 512], BF16, tag="ao", name="ao")
                        nc.vector.scalar_tensor_tensor(
                            ao[:], ps_o[:], 0.0, sr[:], AL.bypass, AL.mult)
                        nblk = 512 // RQ
                        for bi in range(nblk):
                            j = (im * 512) // RQ + bi
                            for dup in (0, 4):
                                nc.sync.dma_start(
                                    a2a_in[hl][(j + dup) * 128:
                                               (j + dup + 1) * 128, :],
                                    ao[:, bi * RQ:(bi + 1) * RQ])
                    nc.gpsimd.collective_compute(
                        "AllToAll", AL.bypass,
                        replica_groups=[list(range(N_CORES))],
                        ins=[a2a_in[hl].opt()],
                        outs=[a2a_out[hl].opt()])
                for hl in range(HL):
                    emit_blend(hl)

              # -------------- Phase C: output projection ----------------
              with (
                tc.tile_pool(name="wos", bufs=8) as wos,
                tc.tile_pool(name="boo", bufs=1) as boo,
                tc.tile_pool(name="stgC", bufs=4) as stgC,
                tc.tile_pool(name="psC", bufs=8, space="PSUM") as psC,
            ):
                bo_row = boo.tile([1, E], F32, name="bo_row")
                nc.sync.dma_start(bo_row[:], bo_d.rearrange("(o q) -> o q", o=1))
                bo_bc = boo.tile([128, E], F32, name="bo_bc")
                for ct in range(4):
                    ps_bo = psC.tile([128, 512], F32, tag="c", name="ps_bo")
                    nc.tensor.matmul(ps_bo[:], ones_row[:],
                                     bo_row[:, ct * 512:(ct + 1) * 512],
                                     start=True, stop=True)
                    nc.scalar.copy(bo_bc[:, ct * 512:(ct + 1) * 512],
                                   ps_bo[:])

                for half in range(2):
                    pos = []
                    for rt in range(RT):
                        for cth in range(2):
                            ps = psC.tile([128, 512], F32, tag="c",
                                          name="ps_c")
                            pos.append(ps)
                    if half == 0:
                        wo_tiles = wo0
                    else:
                        wo_tiles = []
                        for k in range(4 * HL):
                            hl, src = k // 4, k % 4
                            eg = (src * 4 + hl) * 128
                            wt_ = wos.tile([128, 1024], BF16, tag="wo",
                                           name="wok")
                            weng = nc.scalar if k % 2 == 0 else nc.gpsimd
                            weng.dma_start(
                                wt_[:], wo_d[eg:eg + 128, 1024:2048])
                            wo_tiles.append(wt_)
                    for k in range(4 * HL):
                        wt_ = wo_tiles[k]
                        for rt in range(RT):
                            for cth in range(2):
                                nc.tensor.matmul(
                                    pos[rt * 2 + cth][:],
                                    hid[k][:, rt * 128:(rt + 1) * 128],
                                    wt_[:, cth * 512:(cth + 1) * 512],
                                    start=(k == 0), stop=(k == 4 * HL - 1))
                    for rt in range(RT):
                        for cth in range(2):
                            ct = half * 2 + cth
                            so = stgC.tile([128, 512], F32, tag="soC",
                                           name="soC")
                            nc.vector.scalar_tensor_tensor(
                                so[:], pos[rt * 2 + cth][:], 0.0,
                                bo_bc[:, ct * 512:(ct + 1) * 512],
                                AL.bypass, AL.add)
                            nc.sync.dma_start(
                                out_d[rt * 128:(rt + 1) * 128,
                                      ct * 512:(ct + 1) * 512], so[:])

    nc.compile()
    return nc


def make_in_maps(x, Wqkv, bqkv, Wo, bo, seq=SEQ):
    x = np.asarray(x, np.float32)
    Wqkv = np.asarray(Wqkv, np.float32)
    bqkv = np.asarray(bqkv, np.float32)
    import ml_dtypes
    Wo = np.ascontiguousarray(
        np.asarray(Wo, np.float32).astype(ml_dtypes.bfloat16))
    bo = np.asarray(bo, np.float32)
    E = HIDDEN
    ST = seq // 128
    S4 = seq // 512
    slopes = _slopes()
    jp = np.arange(128, dtype=np.float32)
    iif = np.arange(512, dtype=np.float32)
    in_maps = []
    for c in range(N_CORES):
        b, g = c // 4, c % 4
        cols = slice(g * QD, (g + 1) * QD)
        bjv = np.zeros((128, HL * S4 * ST), np.float32)
        bim = np.zeros((128, HL * 512), np.float32)
        for hl in range(HL):
            sl = slopes[g * HL + hl] / SCALE
            for im in range(S4):
                for jt in range(ST):
                    bjv[:, (hl * S4 + im) * ST + jt] = sl * (
                        jt * 128 + jp - im * 512)
            bim[:, hl * 512:(hl + 1) * 512] = -sl * iif[None, :]
        zsel = np.zeros((128, 2), np.float32)
        zsel[:, 0] = 1.0 if b == 0 else 0.0
        zsel[:, 1] = 1.0 - zsel[:, 0]
        in_maps.append({
            "x": np.ascontiguousarray(x[b, :seq]),
            "wq": np.ascontiguousarray(Wqkv[:, cols]),
            "wk": np.ascontiguousarray(Wqkv[:, E + g * QD:E + (g + 1) * QD]),
            "wv": np.ascontiguousarray(
                Wqkv[:, 2 * E + g * QD:2 * E + (g + 1) * QD]),
            "bq": np.ascontiguousarray(bqkv[cols]),
            "bk": np.ascontiguousarray(bqkv[E + g * QD:E + (g + 1) * QD]),
            "bv": np.ascontiguousarray(
                bqkv[2 * E + g * QD:2 * E + (g + 1) * QD]),
            "wo": Wo,
            "bo": bo.copy(),
            "bjv": bjv,
            "bim": bim,
            "zsel": zsel,
        })
    return in_maps


def unshard(outs, seq=SEQ):
    full = np.zeros((BATCH, seq, HIDDEN), np.float32)
    q = seq // 4
    for c in range(N_CORES):
        b, g = c // 4, c % 4
        full[b, g * q:(g + 1) * q, :] = outs[c]["out"]
    return full


_NC_CACHE = {}


def kernel(x, Wqkv, bqkv, Wo, bo):
    key = ("full", SEQ)
    if key not in _NC_CACHE:
        _NC_CACHE[key] = build_nc(SEQ)
    nc = _NC_CACHE[key]
    in_maps = make_in_maps(x, Wqkv, bqkv, Wo, bo)
    res = run_bass_kernel_spmd(nc, in_maps, core_ids=list(range(N_CORES)))
    return unshard(res.results)



# revision 26
# speedup vs baseline: 1.2281x; 1.2281x over previous
"""ALiBi causal attention block (QKV proj + attention + out proj) on 8 TRN2
NeuronCores, written in Bass/Tile. v3: all-SBUF, all-bf16, ALiBi tile skipping.

Sharding: batch(2) x head-group(4) -> 8 cores. Core c (b=c//4, g=c%4) runs
heads {(3-hl)*4+g : hl in 0..3} of its batch: slot hl on every core holds a
head from the same slope quartile, so the causal+ALiBi tile-skip pattern is
identical across cores (one SPMD program) and balanced. QKV projection and
attention are comm-free per core. A per-head 8-core AllToAll (bf16)
redistributes attention outputs from head-sharding to row-sharding; a zsel
0/1 blend drops the duplicate cross-batch shards. Each core then multiplies
its 512 output rows by the full Wo (bf16) and writes rows
[512g, 512(g+1)) of its batch.

v3 vs the DRAM-roundtrip baseline:
- q/k/v stay SBUF-resident between projection and attention (bf16); no
  per-head DRAM reloads in phase B.
- All GEMMs in bf16 (1 cycle/row on the PE); x transposed in fp32r and cast
  to bf16 on the PSUM->SBUF copy (ACT).
- ALiBi decays by ~e^-25 within B_SLOT tiles of the diagonal, so far
  below-diagonal score tiles are skipped: 121 of 160 tiles per core.
- Bias+mask tiles precomputed on host (bf16), DMA'd in; the gpsimd queue
  holds only collective triggers + half the bias-add STTs, so each head's
  AllToAll fires right after its outputs ship.
- Phase-B bias adds alternate DVE/Pool; exps on ACT; the PE stream is
  software-pipelined 2 deep so it never waits on the softmax chain.
- Transposes for chunk s4+1 emitted interleaved between chunk s4's
  projection chains.
"""

import math
from collections import deque

import numpy as np

import concourse.bass as bass
import concourse.mybir as mybir
import concourse.tile as tile
from concourse import bacc
from concourse.bass_utils import run_bass_kernel_spmd
from concourse.masks import make_identity

F32 = mybir.dt.float32
F32R = mybir.dt.float32r
BF16 = mybir.dt.bfloat16
AL = mybir.AluOpType
AF = mybir.ActivationFunctionType

HIDDEN = 2048
NUM_HEADS = 16
HEAD = 128
SEQ = 2048
BATCH = 2
N_CORES = 8
HL = 4                      # heads per core
QD = HL * HEAD              # 512 projected cols per core
SCALE = 1.0 / math.sqrt(HEAD)
NEG = -1.0e6
ST = SEQ // 128             # 16 seq tiles
S4 = SEQ // 512             # 4 coarse chunks
ET = HIDDEN // 128          # 16 contraction tiles
RQ = SEQ // 4               # 512 output rows per core
RT = RQ // 128              # 4

# Below-diagonal tiles kept per slot: keep jt >= 4*im - B_SLOT[hl]. Slot hl
# holds head (3-hl)*4+g, so slot 0 has the smallest slopes (keep everything)
# and slot 3 the largest (keep only 1 below-diagonal tile).
B_SLOT = [16, 9, 3, 1]


def head_of(g, hl):
    return (3 - hl) * 4 + g


def _slopes():
    if NUM_HEADS <= 8:
        return [1.0 / 2 ** k for k in range(NUM_HEADS)]
    return [1.0 / 2 ** (k / 2) for k in range(NUM_HEADS)]


def _r(ap):
    return ap.bitcast(F32R)


def build_nc(seq=SEQ):
    E = HIDDEN

    nc = bacc.Bacc("TRN2", target_bir_lowering=False, debug=False,
                   num_devices=N_CORES)

    x_d = nc.dram_tensor("x", [seq, E], F32, kind="ExternalInput").ap()
    wq_d = nc.dram_tensor("wq", [E, QD], BF16, kind="ExternalInput").ap()
    wk_d = nc.dram_tensor("wk", [E, QD], BF16, kind="ExternalInput").ap()
    wv_d = nc.dram_tensor("wv", [E, QD], BF16, kind="ExternalInput").ap()
    bq_d = nc.dram_tensor("bq", [QD], F32, kind="ExternalInput").ap()
    bk_d = nc.dram_tensor("bk", [QD], F32, kind="ExternalInput").ap()
    bv_d = nc.dram_tensor("bv", [QD], F32, kind="ExternalInput").ap()
    wo_d = nc.dram_tensor("wo", [E, E], BF16, kind="ExternalInput").ap()
    bo_d = nc.dram_tensor("bo", [E], F32, kind="ExternalInput").ap()
    # slots 2,3 (large slopes): pre-exp bias tables (STT path)
    bjv_d = nc.dram_tensor("bjv", [128, 2 * S4 * ST], F32,
                           kind="ExternalInput").ap()
    bim_d = nc.dram_tensor("bim", [128, 2 * 512], BF16,
                           kind="ExternalInput").ap()
    bmask_d = nc.dram_tensor("bmask", [128, 2 * 4 * 512], BF16,
                             kind="ExternalInput").ap()
    # slots 0,1 (small slopes): factorized form. The per-column factor
    # exp(-sl*i_loc) cancels in the softmax normalization, so only the
    # 0/1 causal pattern (head-independent) is applied post-exp.
    bjv2_d = nc.dram_tensor("bjv2", [128, 2 * S4 * ST], F32,
                            kind="ExternalInput").ap()
    cau_d = nc.dram_tensor("cau", [128, 4 * 512], BF16,
                           kind="ExternalInput").ap()
    zsel_d = nc.dram_tensor("zsel", [128, 2], F32, kind="ExternalInput").ap()
    out_d = nc.dram_tensor("out", [RQ, E], F32, kind="ExternalOutput").ap()

    with tile.TileContext(nc) as tc:
        with (
            tc.tile_pool(name="const", bufs=1) as cpool,
            tc.tile_pool(name="persist", bufs=1) as pers,
            tc.tile_pool(name="dram", bufs=1, space="DRAM") as dpool,
            tc.tile_pool(name="psum", bufs=1, space="PSUM") as psum,
        ):
            # ---------------- constants ----------------
            ident = cpool.tile([128, 128], F32, name="ident")
            make_identity(nc, ident[:])
            ident_r = cpool.tile([128, 128], F32R, name="ident_r")
            nc.vector.tensor_copy(ident_r[:], ident[:])
            ones_col = cpool.tile([128, 1], BF16, name="ones_col")
            nc.gpsimd.memset(ones_col[:], 1.0)
            ones_row = cpool.tile([1, 128], F32, name="ones_row")
            nc.gpsimd.memset(ones_row[:], 1.0)
            ones_row_r = cpool.tile([1, 128], F32R, name="ones_row_r")
            nc.vector.tensor_copy(ones_row_r[:], ones_row[:])
            zsel = cpool.tile([128, 2], F32, name="zsel")
            nc.sync.dma_start(zsel[:], zsel_d[:])

            # persistent SBUF state
            qT_sb = [pers.tile([128, seq], BF16, name=f"qT{h}")
                     for h in range(HL)]
            kT_sb = [pers.tile([128, seq], BF16, name=f"kT{h}")
                     for h in range(HL)]
            v_sb = [pers.tile([128, QD], BF16, name=f"v{st}")
                    for st in range(ST)]
            bv_bc = pers.tile([128, QD], F32, name="bv_bc")

            # bias/mask tables (loaded during phase A on the scalar queue)
            bjv = pers.tile([128, 2 * S4 * ST], F32, name="bjv")
            nc.scalar.dma_start(bjv[:], bjv_d[:])
            bim = pers.tile([128, 2 * 512], BF16, name="bim")
            nc.scalar.dma_start(bim[:], bim_d[:])
            bmask = pers.tile([128, 2 * 4 * 512], BF16, name="bmask")
            nc.scalar.dma_start(bmask[:], bmask_d[:])
            bjv2 = pers.tile([128, 2 * S4 * ST], F32, name="bjv2")
            nc.scalar.dma_start(bjv2[:], bjv2_d[:])
            cau = pers.tile([128, 4 * 512], BF16, name="cau")
            nc.scalar.dma_start(cau[:], cau_d[:])

            a2a_in = [dpool.tile([N_CORES * 128, RQ], BF16, name=f"a2ai{h}")
                      for h in range(HL)]
            a2a_out = [dpool.tile([N_CORES * 128, RQ], BF16, name=f"a2ao{h}")
                       for h in range(HL)]

            # ---------------- Phase A: QKV projection ----------------
            with (
                tc.tile_pool(name="wp", bufs=1) as wp,
                tc.tile_pool(name="xp", bufs=4) as xp,
                tc.tile_pool(name="xtp", bufs=2) as xtp,
            ):
                # bv broadcast row: borrow an xp ring slot transiently
                # (a [1,N] tile pads to 128 partitions, so a dedicated pool
                # would waste SBUF).
                brow = xp.tile([1, E], F32R, tag="xn", name="brow")
                nc.sync.dma_start(brow[:, :QD],
                                  _r(bv_d.rearrange("(o q) -> o q", o=1)))
                ps_bv = psum.tile([128, 512], F32, tag="bc", bufs=1,
                                  name="ps_bv")
                nc.tensor.matmul(ps_bv[:], ones_row_r[:],
                                 brow[:, :QD], start=True, stop=True)
                nc.scalar.copy(bv_bc[:], ps_bv[:])
                wt = {}
                for wi, wd in enumerate((wq_d, wk_d, wv_d)):
                    for et in range(ET):
                        t = wp.tile([128, QD], BF16, name=f"w{wi}_{et}")
                        nc.scalar.dma_start(
                            t[:], wd[et * 128:(et + 1) * 128, :])
                        wt[(wi, et)] = t
                bvec = {}
                for bi, bd in enumerate((bq_d, bk_d)):
                    for m in range(HL):
                        t = cpool.tile([128, 1], F32, name=f"b{bi}_{m}")
                        nc.sync.dma_start(
                            t[:], bd[m * 128:(m + 1) * 128].rearrange(
                                "(p o) -> p o", o=1))
                        bvec[(bi, m)] = t

                xtiles = {}

                def load_x(s4):
                    for st in range(4):
                        t = xp.tile([128, E], F32R, tag="xn", name="xn")
                        nc.sync.dma_start(
                            t[:], _r(x_d[(s4 * 4 + st) * 128:
                                         (s4 * 4 + st + 1) * 128, :]))
                        xtiles[(s4, st)] = t

                xT = {}

                def emit_transpose(s4, et):
                    # 4 transposes of [128,128] into one psum tile's quarters,
                    # then a single ACT copy casting to bf16.
                    pt = psum.tile([128, 512], F32R, tag="o", bufs=2,
                                   name="ps_tp")
                    for st in range(4):
                        nc.tensor.transpose(
                            pt[:, st * 128:(st + 1) * 128],
                            xtiles[(s4, st)][:, et * 128:(et + 1) * 128],
                            ident_r[:])
                    t = xtp.tile([128, 512], BF16, tag=f"xT{et}",
                                 name=f"xT{et}")
                    nc.scalar.copy(t[:], pt[:].bitcast(F32))
                    xT[(s4, et)] = t

                load_x(0)
                for et in range(ET):
                    emit_transpose(0, et)

                for s4 in range(S4):
                    if s4 + 1 < S4:
                        load_x(s4 + 1)
                    tp_next = list(range(ET)) if s4 + 1 < S4 else []

                    def chain_qk(wi, m):
                        ps = psum.tile([128, 512], F32, tag="mm", bufs=3,
                                       name="ps_mm")
                        for et in range(ET):
                            nc.tensor.matmul(
                                ps[:],
                                wt[(wi, et)][:, m * 128:(m + 1) * 128],
                                xT[(s4, et)][:],
                                start=(et == 0), stop=(et == ET - 1))
                        dst = qT_sb[m] if wi == 0 else kT_sb[m]
                        nc.scalar.activation(
                            dst[:, s4 * 512:(s4 + 1) * 512], ps[:],
                            AF.Identity, bias=bvec[(wi, m)][:], scale=1.0)

                    def chain_v(st):
                        ps = psum.tile([128, 512], F32, tag="mm", bufs=3,
                                       name="ps_mv")
                        for et in range(ET):
                            nc.tensor.matmul(
                                ps[:],
                                xT[(s4, et)][:, st * 128:(st + 1) * 128],
                                wt[(2, et)][:],
                                start=(et == 0), stop=(et == ET - 1))
                        nc.vector.scalar_tensor_tensor(
                            v_sb[s4 * 4 + st][:], ps[:], 0.0, bv_bc[:],
                            AL.bypass, AL.add)

                    ci = 0
                    for wi in (0, 1):
                        for m in range(HL):
                            chain_qk(wi, m)
                            while len(tp_next) > (11 - ci) * ET // 12:
                                emit_transpose(s4 + 1, tp_next.pop(0))
                            ci += 1
                    for st in range(4):
                        chain_v(st)
                        while len(tp_next) > (11 - ci) * ET // 12:
                            emit_transpose(s4 + 1, tp_next.pop(0))
                        ci += 1

            # ---------------- Phase B: attention ----------------
            with (
                tc.tile_pool(name="wop", bufs=1) as wop,
                tc.tile_pool(name="hidp", bufs=1) as hidp,
                tc.tile_pool(name="bop", bufs=1) as bop,
                tc.tile_pool(name="pp", bufs=6) as ppool,
                tc.tile_pool(name="stgB", bufs=3) as stgB,
                tc.tile_pool(name="aop", bufs=3) as aop,
                tc.tile_pool(name="ldp", bufs=4) as ldp,
                tc.tile_pool(name="blt", bufs=2) as blt,
            ):
                hid = [hidp.tile([128, RQ], BF16, name=f"hid{k}")
                       for k in range(4 * HL)]
                bo_bc = bop.tile([128, E], F32, name="bo_bc")
                bo_row = bop.tile([1, E], F32R, name="bo_row")
                nc.sync.dma_start(bo_row[:],
                                  _r(bo_d.rearrange("(o q) -> o q", o=1)))
                for ct in range(4):
                    ps_bo = psum.tile([128, 512], F32, tag="bc", bufs=1,
                                      name="ps_bo")
                    nc.tensor.matmul(
                        ps_bo[:], ones_row_r[:],
                        bo_row[:, ct * 512:(ct + 1) * 512],
                        start=True, stop=True)
                    nc.scalar.copy(bo_bc[:, ct * 512:(ct + 1) * 512],
                                   ps_bo[:])
                # Wo half-0 prefetch (cols 0:1024), used by phase C.
                wo0 = []
                for k in range(4 * HL):
                    hl_, src_ = k // 4, k % 4
                    eg = head_of(src_, hl_) * 128
                    t = wop.tile([128, 1024], BF16, tag="wo", name="wok",
                                 bufs=16)
                    nc.sync.dma_start(t[:], wo_d[eg:eg + 128, 0:1024])
                    wo0.append(t)

                pending_cc = [None]

                def emit_cc(hl):
                    nc.gpsimd.collective_compute(
                        "AllToAll", AL.bypass,
                        replica_groups=[list(range(N_CORES))],
                        ins=[a2a_in[hl].opt()],
                        outs=[a2a_out[hl].opt()])
                    for src_ in range(4):
                        k = hl * 4 + src_
                        la = ldp.tile([128, RQ], BF16, tag="la", name="la")
                        nc.sync.dma_start(
                            la[:],
                            a2a_out[hl][src_ * 128:(src_ + 1) * 128, :])
                        lb = ldp.tile([128, RQ], BF16, tag="lb", name="lb")
                        nc.sync.dma_start(
                            lb[:],
                            a2a_out[hl][(src_ + 4) * 128:(src_ + 5) * 128, :])
                        tmp = blt.tile([128, RQ], BF16, tag="tmp", name="tmp")
                        nc.vector.tensor_scalar(
                            tmp[:], lb[:], zsel[:, 1:2], None, AL.mult)
                        nc.vector.scalar_tensor_tensor(
                            hid[k][:], la[:], zsel[:, 0:1], tmp[:],
                            AL.mult, AL.add)

                for hl in range(HL):
                    for im in range(S4):
                        njt = 4 * im + 4
                        kept = [jt for jt in range(njt)
                                if jt >= 4 * im - B_SLOT[hl]]
                        first, last = kept[0], kept[-1]
                        ps_o = psum.tile([128, 512], F32, tag="o", bufs=2,
                                         name="ps_o")
                        ps_d = psum.tile([1, 512], F32, tag="d", bufs=2,
                                         name="ps_d")

                        def consume(jt_, p_):
                            nc.tensor.matmul(ps_d[:], ones_col[:], p_[:],
                                             start=(jt_ == first),
                                             stop=(jt_ == last))
                            nc.tensor.matmul(
                                ps_o[:],
                                v_sb[jt_][:, hl * 128:(hl + 1) * 128], p_[:],
                                start=(jt_ == first), stop=(jt_ == last))

                        pipe = deque()
                        for ji, jt in enumerate(kept):
                            ps_s = psum.tile([128, 512], F32, tag="mm",
                                             bufs=3, name="ps_s")
                            nc.tensor.matmul(
                                ps_s[:],
                                kT_sb[hl][:, jt * 128:(jt + 1) * 128],
                                qT_sb[hl][:, im * 512:(im + 1) * 512],
                                start=True, stop=True)
                            r = jt - 4 * im
                            p = ppool.tile([128, 512], BF16, tag="p",
                                           name="p")
                            if hl < 2:
                                # factorized: exp(scale*s + sl*(j_loc+d0));
                                # the per-column exp(-sl*i_loc) factor
                                # cancels in the normalization, so only the
                                # 0/1 causal pattern is applied on diagonal
                                # tiles.
                                idx = (hl * S4 + im) * ST + jt
                                jv2 = bjv2[:, idx:idx + 1]
                                nc.scalar.activation(p[:], ps_s[:], AF.Exp,
                                                     bias=jv2,
                                                     scale=SCALE)
                                if r >= 0:
                                    nc.vector.tensor_tensor(
                                        p[:], p[:],
                                        cau[:, r * 512:(r + 1) * 512],
                                        AL.mult)
                            else:
                                # large slopes: pre-exp bias+mask add (DVE,
                                # PSUM) then plain exp.
                                h2 = hl - 2
                                idx = (h2 * S4 + im) * ST + jt
                                jv = bjv[:, idx:idx + 1]
                                if r >= 0:
                                    in1 = bmask[:, (h2 * 4 + r) * 512:
                                                (h2 * 4 + r + 1) * 512]
                                else:
                                    in1 = bim[:, h2 * 512:(h2 + 1) * 512]
                                nc.vector.scalar_tensor_tensor(
                                    ps_s[:], ps_s[:], jv, in1,
                                    AL.add, AL.add)
                                nc.scalar.activation(p[:], ps_s[:], AF.Exp,
                                                     scale=SCALE)
                            pipe.append((jt, p))
                            if len(pipe) > 2:
                                consume(*pipe.popleft())
                        while pipe:
                            consume(*pipe.popleft())

                        sd = stgB.tile([1, 512], F32R, tag="sd", name="sd")
                        nc.vector.tensor_copy(sd[:], ps_d[:])
                        ps_b = psum.tile([128, 512], F32, tag="bc", bufs=1,
                                         name="ps_b")
                        nc.tensor.matmul(ps_b[:], ones_row_r[:], sd[:],
                                         start=True, stop=True)
                        sr = stgB.tile([128, 512], F32, tag="sr", name="sr")
                        nc.vector.reciprocal_approx_fast(sr[:], ps_b[:])
                        ao = aop.tile([128, 512], BF16, tag="ao", name="ao")
                        nc.vector.scalar_tensor_tensor(
                            ao[:], ps_o[:], 0.0, sr[:], AL.bypass, AL.mult)
                        for dup in (0, 4):
                            nc.sync.dma_start(
                                a2a_in[hl][(im + dup) * 128:
                                           (im + dup + 1) * 128, :],
                                ao[:])
                        if im == 0 and pending_cc[0] is not None:
                            emit_cc(pending_cc[0])
                            pending_cc[0] = None
                    pending_cc[0] = hl
                emit_cc(pending_cc[0])

                # -------------- Phase C: output projection --------------
                with tc.tile_pool(name="stgC", bufs=4) as stgC:
                    acc_spec = [("mm", 3), ("mm", 3), ("mm", 3), ("o", 2),
                                ("o", 2), ("d", 2), ("d", 2), ("bc", 1)]
                    for half in range(2):
                        if half == 0:
                            wo_tiles = wo0
                        else:
                            wo_tiles = []
                            for k in range(4 * HL):
                                hl_, src_ = k // 4, k % 4
                                eg = head_of(src_, hl_) * 128
                                t = wop.tile([128, 1024], BF16, tag="wo",
                                             name="wok2", bufs=16)
                                nc.sync.dma_start(
                                    t[:], wo_d[eg:eg + 128, 1024:2048])
                                wo_tiles.append(t)
                        pos = [psum.tile([128, 512], F32, tag=tg, bufs=bf_,
                                         name="ps_c")
                               for tg, bf_ in acc_spec]
                        for k in range(4 * HL):
                            wt_ = wo_tiles[k]
                            for rt in range(RT):
                                for cth in range(2):
                                    nc.tensor.matmul(
                                        pos[rt * 2 + cth][:],
                                        hid[k][:, rt * 128:(rt + 1) * 128],
                                        wt_[:, cth * 512:(cth + 1) * 512],
                                        start=(k == 0), stop=(k == 4 * HL - 1))
                        for rt in range(RT):
                            for cth in range(2):
                                ct = half * 2 + cth
                                so = stgC.tile([128, 512], F32, tag="soC",
                                               name="soC")
                                nc.vector.scalar_tensor_tensor(
                                    so[:], pos[rt * 2 + cth][:], 0.0,
                                    bo_bc[:, ct * 512:(ct + 1) * 512],
                                    AL.bypass, AL.add)
                                nc.sync.dma_start(
                                    out_d[rt * 128:(rt + 1) * 128,
                                          ct * 512:(ct + 1) * 512], so[:])

    nc.compile()
    return nc


def make_in_maps(x, Wqkv, bqkv, Wo, bo, seq=SEQ):
    import ml_dtypes
    x = np.asarray(x, np.float32)
    Wqkv = np.asarray(Wqkv, np.float32)
    bqkv = np.asarray(bqkv, np.float32)
    Wo = np.ascontiguousarray(
        np.asarray(Wo, np.float32).astype(ml_dtypes.bfloat16))
    bo = np.asarray(bo, np.float32)
    E = HIDDEN
    slopes = _slopes()
    jp = np.arange(128, dtype=np.float32)
    iif = np.arange(512, dtype=np.float32)
    bf16 = ml_dtypes.bfloat16
    in_maps = []
    for c in range(N_CORES):
        b, g = c // 4, c % 4
        heads = [head_of(g, hl) for hl in range(HL)]
        hcols = np.concatenate(
            [np.arange(h * HEAD, (h + 1) * HEAD) for h in heads])
        bjv = np.zeros((128, 2 * S4 * ST), np.float32)
        bim = np.zeros((128, 2 * 512), np.float32)
        bmask = np.zeros((128, 2 * 4 * 512), np.float32)
        bjv2 = np.zeros((128, 2 * S4 * ST), np.float32)
        cau = np.zeros((128, 4 * 512), np.float32)
        for r in range(4):
            cau[:, r * 512:(r + 1) * 512] = (
                iif[None, :] >= (128 * r + jp[:, None])).astype(np.float32)
        for hl in range(HL):
            sl_pre = slopes[heads[hl]] / SCALE   # pre-scale units
            sl = slopes[heads[hl]]               # post-scale units
            if hl < 2:
                # factorized path: column factor dropped (cancels in the
                # softmax normalization)
                for im in range(S4):
                    for jt in range(ST):
                        # bias (post-scale): sl*(j_loc + 128*jt - 512*im)
                        bjv2[:, (hl * S4 + im) * ST + jt] = sl * (
                            jp + 128 * jt - 512 * im)
            else:
                h2 = hl - 2
                for im in range(S4):
                    for jt in range(ST):
                        bjv[:, (h2 * S4 + im) * ST + jt] = sl_pre * (
                            jt * 128 + jp - im * 512)
                bim[:, h2 * 512:(h2 + 1) * 512] = -sl_pre * iif[None, :]
                for r in range(4):
                    blk = bmask[:, (h2 * 4 + r) * 512:(h2 * 4 + r + 1) * 512]
                    blk[:] = -sl_pre * iif[None, :]
                    keep = iif[None, :] >= (128 * r + jp[:, None])
                    blk[~keep] = NEG
        zsel = np.zeros((128, 2), np.float32)
        zsel[:, 0] = 1.0 if b == 0 else 0.0
        zsel[:, 1] = 1.0 - zsel[:, 0]
        castw = lambda a: np.ascontiguousarray(a.astype(bf16))
        in_maps.append({
            "x": np.ascontiguousarray(x[b, :seq]),
            "wq": castw(Wqkv[:, hcols]),
            "wk": castw(Wqkv[:, E + hcols]),
            "wv": castw(Wqkv[:, 2 * E + hcols]),
            "bq": np.ascontiguousarray(bqkv[hcols]),
            "bk": np.ascontiguousarray(bqkv[E + hcols]),
            "bv": np.ascontiguousarray(bqkv[2 * E + hcols]),
            "wo": Wo,
            "bo": bo.copy(),
            "bjv": bjv,
            "bim": np.ascontiguousarray(bim.astype(bf16)),
            "bmask": np.ascontiguousarray(bmask.astype(bf16)),
            "bjv2": bjv2,
            "cau": np.ascontiguousarray(cau.astype(bf16)),
            "zsel": zsel,
        })
    return in_maps


def unshard(outs, seq=SEQ):
    full = np.zeros((BATCH, seq, HIDDEN), np.float32)
    q = seq // 4
    for c in range(N_CORES):
        b, g = c // 4, c % 4
        full[b, g * q:(g + 1) * q, :] = outs[c]["out"]
    return full


_NC_CACHE = {}


def kernel(x, Wqkv, bqkv, Wo, bo):
    key = ("full", SEQ)
    if key not in _NC_CACHE:
        _NC_CACHE[key] = build_nc(SEQ)
    nc = _NC_CACHE[key]
    in_maps = make_in_maps(x, Wqkv, bqkv, Wo, bo)
    res = run_bass_kernel_spmd(nc, in_maps, core_ids=list(range(N_CORES)))
    return unshard(res.results)


# revision 34
# speedup vs baseline: 1.2367x; 1.0070x over previous
"""ALiBi causal attention block (QKV proj + attention + out proj) on 8 TRN2
NeuronCores, written in Bass/Tile. v3: all-SBUF, all-bf16, ALiBi tile skipping.

Sharding: batch(2) x head-group(4) -> 8 cores. Core c (b=c//4, g=c%4) runs
heads {(3-hl)*4+g : hl in 0..3} of its batch: slot hl on every core holds a
head from the same slope quartile, so the causal+ALiBi tile-skip pattern is
identical across cores (one SPMD program) and balanced. QKV projection and
attention are comm-free per core. A per-head 8-core AllToAll (bf16)
redistributes attention outputs from head-sharding to row-sharding; a zsel
0/1 blend drops the duplicate cross-batch shards. Each core then multiplies
its 512 output rows by the full Wo (bf16) and writes rows
[512g, 512(g+1)) of its batch.

v3 vs the DRAM-roundtrip baseline:
- q/k/v stay SBUF-resident between projection and attention (bf16); no
  per-head DRAM reloads in phase B.
- All GEMMs in bf16 (1 cycle/row on the PE); x transposed in fp32r and cast
  to bf16 on the PSUM->SBUF copy (ACT).
- ALiBi decays by ~e^-25 within B_SLOT tiles of the diagonal, so far
  below-diagonal score tiles are skipped: 121 of 160 tiles per core.
- Bias+mask tiles precomputed on host (bf16), DMA'd in; the gpsimd queue
  holds only collective triggers + half the bias-add STTs, so each head's
  AllToAll fires right after its outputs ship.
- Phase-B bias adds alternate DVE/Pool; exps on ACT; the PE stream is
  software-pipelined 2 deep so it never waits on the softmax chain.
- Transposes for chunk s4+1 emitted interleaved between chunk s4's
  projection chains.
"""

import math
from collections import deque

import numpy as np

import concourse.bass as bass
import concourse.mybir as mybir
import concourse.tile as tile
from concourse import bacc
from concourse.bass_utils import run_bass_kernel_spmd
from concourse.masks import make_identity

F32 = mybir.dt.float32
F32R = mybir.dt.float32r
BF16 = mybir.dt.bfloat16
AL = mybir.AluOpType
AF = mybir.ActivationFunctionType

HIDDEN = 2048
NUM_HEADS = 16
HEAD = 128
SEQ = 2048
BATCH = 2
N_CORES = 8
HL = 4                      # heads per core
QD = HL * HEAD              # 512 projected cols per core
SCALE = 1.0 / math.sqrt(HEAD)
NEG = -1.0e6
ST = SEQ // 128             # 16 seq tiles
S4 = SEQ // 512             # 4 coarse chunks
ET = HIDDEN // 128          # 16 contraction tiles
RQ = SEQ // 4               # 512 output rows per core
RT = RQ // 128              # 4

# Below-diagonal tiles kept per slot: keep jt >= 4*im - B_SLOT[hl]. Slot hl
# holds head (3-hl)*4+g, so slot 0 has the smallest slopes (keep everything)
# and slot 3 the largest (keep only 1 below-diagonal tile).
B_SLOT = [16, 9, 3, 1]


def head_of(g, hl):
    return (3 - hl) * 4 + g


def _slopes():
    if NUM_HEADS <= 8:
        return [1.0 / 2 ** k for k in range(NUM_HEADS)]
    return [1.0 / 2 ** (k / 2) for k in range(NUM_HEADS)]


def _r(ap):
    return ap.bitcast(F32R)


def build_nc(seq=SEQ):
    E = HIDDEN

    nc = bacc.Bacc("TRN2", target_bir_lowering=False, debug=False,
                   num_devices=N_CORES)

    x_d = nc.dram_tensor("x", [seq, E], F32, kind="ExternalInput").ap()
    wq_d = nc.dram_tensor("wq", [E, QD], BF16, kind="ExternalInput").ap()
    wk_d = nc.dram_tensor("wk", [E, QD], BF16, kind="ExternalInput").ap()
    wv_d = nc.dram_tensor("wv", [E, QD], BF16, kind="ExternalInput").ap()
    bq_d = nc.dram_tensor("bq", [QD], F32, kind="ExternalInput").ap()
    bk_d = nc.dram_tensor("bk", [QD], F32, kind="ExternalInput").ap()
    bv_d = nc.dram_tensor("bv", [QD], F32, kind="ExternalInput").ap()
    wo_d = nc.dram_tensor("wo", [E, E], BF16, kind="ExternalInput").ap()
    bo_d = nc.dram_tensor("bo", [E], F32, kind="ExternalInput").ap()
    # slot 3 (largest slopes): pre-exp bias tables (STT path)
    bjv_d = nc.dram_tensor("bjv", [128, S4 * ST], F32,
                           kind="ExternalInput").ap()
    bim_d = nc.dram_tensor("bim", [128, 512], BF16,
                           kind="ExternalInput").ap()
    bmask_d = nc.dram_tensor("bmask", [128, 4 * 512], BF16,
                             kind="ExternalInput").ap()
    # slots 0-2: factorized form. The per-column factor exp(-sl*(i-anchor))
    # cancels in the softmax normalization, so only the 0/1 causal pattern
    # (head-independent) is applied post-exp.
    bjv2_d = nc.dram_tensor("bjv2", [128, 3 * S4 * ST], F32,
                            kind="ExternalInput").ap()
    cau_d = nc.dram_tensor("cau", [128, 4 * 512], BF16,
                           kind="ExternalInput").ap()
    zsel_d = nc.dram_tensor("zsel", [128, 2], F32, kind="ExternalInput").ap()
    out_d = nc.dram_tensor("out", [RQ, E], F32, kind="ExternalOutput").ap()

    with tile.TileContext(nc) as tc:
        with (
            tc.tile_pool(name="const", bufs=1) as cpool,
            tc.tile_pool(name="persist", bufs=1) as pers,
            tc.tile_pool(name="dram", bufs=1, space="DRAM") as dpool,
            tc.tile_pool(name="psum", bufs=1, space="PSUM") as psum,
        ):
            # ---------------- constants ----------------
            ident = cpool.tile([128, 128], F32, name="ident")
            make_identity(nc, ident[:])
            ident_r = cpool.tile([128, 128], F32R, name="ident_r")
            nc.vector.tensor_copy(ident_r[:], ident[:])
            ones_col = cpool.tile([128, 1], BF16, name="ones_col")
            nc.gpsimd.memset(ones_col[:], 1.0)
            ones_row = cpool.tile([1, 128], F32, name="ones_row")
            nc.gpsimd.memset(ones_row[:], 1.0)
            ones_row_r = cpool.tile([1, 128], F32R, name="ones_row_r")
            nc.vector.tensor_copy(ones_row_r[:], ones_row[:])
            zsel = cpool.tile([128, 2], F32, name="zsel")
            nc.sync.dma_start(zsel[:], zsel_d[:])

            # persistent SBUF state
            qT_sb = [pers.tile([128, seq], BF16, name=f"qT{h}")
                     for h in range(HL)]
            kT_sb = [pers.tile([128, seq], BF16, name=f"kT{h}")
                     for h in range(HL)]
            v_sb = [pers.tile([128, QD], BF16, name=f"v{st}")
                    for st in range(ST)]
            bv_bc = pers.tile([128, QD], F32, name="bv_bc")

            # bias/mask tables (loaded on sync, after x chunk 0 and W; the
            # scalar/ACT queue stays clear of DMA-trigger instructions)
            bias_tables = [
                (pers.tile([128, S4 * ST], F32, name="bjv"), bjv_d),
                (pers.tile([128, 512], BF16, name="bim"), bim_d),
                (pers.tile([128, 4 * 512], BF16, name="bmask"), bmask_d),
                (pers.tile([128, 3 * S4 * ST], F32, name="bjv2"), bjv2_d),
                (pers.tile([128, 4 * 512], BF16, name="cau"), cau_d),
            ]
            bjv, bim, bmask, bjv2, cau = (t for t, _ in bias_tables)

            a2a_in = [dpool.tile([N_CORES * 128, RQ], BF16, name=f"a2ai{h}")
                      for h in range(HL)]
            a2a_out = [dpool.tile([N_CORES * 128, RQ], BF16, name=f"a2ao{h}")
                       for h in range(HL)]

            # ---------------- Phase A: QKV projection ----------------
            with (
                tc.tile_pool(name="wp", bufs=1) as wp,
                tc.tile_pool(name="xp", bufs=4) as xp,
                tc.tile_pool(name="xtp", bufs=2) as xtp,
            ):
                # bv broadcast row: borrow an xp ring slot transiently
                # (a [1,N] tile pads to 128 partitions, so a dedicated pool
                # would waste SBUF).
                brow = xp.tile([1, E], F32R, tag="xn", name="brow")
                nc.sync.dma_start(brow[:, :QD],
                                  _r(bv_d.rearrange("(o q) -> o q", o=1)))
                ps_bv = psum.tile([128, 512], F32, tag="bc", bufs=1,
                                  name="ps_bv")
                nc.tensor.matmul(ps_bv[:], ones_row_r[:],
                                 brow[:, :QD], start=True, stop=True)
                nc.scalar.copy(bv_bc[:], ps_bv[:])
                xtiles = {}

                def load_x(s4):
                    for st in range(4):
                        t = xp.tile([128, E], F32R, tag="xn", name="xn")
                        nc.sync.dma_start(
                            t[:], _r(x_d[(s4 * 4 + st) * 128:
                                         (s4 * 4 + st + 1) * 128, :]))
                        xtiles[(s4, st)] = t

                load_x(0)
                wt = {}
                for wi, wd in enumerate((wq_d, wk_d, wv_d)):
                    for et in range(ET):
                        t = wp.tile([128, QD], BF16, name=f"w{wi}_{et}")
                        nc.sync.dma_start(
                            t[:], wd[et * 128:(et + 1) * 128, :])
                        wt[(wi, et)] = t
                bvec = {}
                for bi, bd in enumerate((bq_d, bk_d)):
                    for m in range(HL):
                        t = cpool.tile([128, 1], F32, name=f"b{bi}_{m}")
                        nc.sync.dma_start(
                            t[:], bd[m * 128:(m + 1) * 128].rearrange(
                                "(p o) -> p o", o=1))
                        bvec[(bi, m)] = t
                for t, td in bias_tables:
                    nc.sync.dma_start(t[:], td[:])

                xT = {}

                def emit_transpose(s4, et):
                    # 4 transposes of [128,128] into one psum tile's quarters,
                    # then a single DVE copy casting to bf16.
                    pt = psum.tile([128, 512], F32R, tag="o", bufs=2,
                                   name="ps_tp")
                    for st in range(4):
                        nc.tensor.transpose(
                            pt[:, st * 128:(st + 1) * 128],
                            xtiles[(s4, st)][:, et * 128:(et + 1) * 128],
                            ident_r[:])
                    t = xtp.tile([128, 512], BF16, tag=f"xT{et}",
                                 name=f"xT{et}")
                    nc.vector.tensor_copy(t[:], pt[:].bitcast(F32))
                    xT[(s4, et)] = t

                for et in range(ET):
                    emit_transpose(0, et)

                for s4 in range(S4):
                    if s4 + 1 < S4:
                        load_x(s4 + 1)
                    tp_next = list(range(ET)) if s4 + 1 < S4 else []

                    def chain_qk(wi, m):
                        ps = psum.tile([128, 512], F32, tag="mm", bufs=4,
                                       name="ps_mm")
                        for et in range(ET):
                            nc.tensor.matmul(
                                ps[:],
                                wt[(wi, et)][:, m * 128:(m + 1) * 128],
                                xT[(s4, et)][:],
                                start=(et == 0), stop=(et == ET - 1))
                        dst = qT_sb[m] if wi == 0 else kT_sb[m]
                        nc.scalar.activation(
                            dst[:, s4 * 512:(s4 + 1) * 512], ps[:],
                            AF.Identity, bias=bvec[(wi, m)][:], scale=1.0)

                    def chain_v(st):
                        ps = psum.tile([128, 512], F32, tag="mm", bufs=4,
                                       name="ps_mv")
                        for et in range(ET):
                            nc.tensor.matmul(
                                ps[:],
                                xT[(s4, et)][:, st * 128:(st + 1) * 128],
                                wt[(2, et)][:],
                                start=(et == 0), stop=(et == ET - 1))
                        nc.vector.scalar_tensor_tensor(
                            v_sb[s4 * 4 + st][:], ps[:], 0.0, bv_bc[:],
                            AL.bypass, AL.add)

                    ci = 0
                    for wi in (0, 1):
                        for m in range(HL):
                            chain_qk(wi, m)
                            while len(tp_next) > (11 - ci) * ET // 12:
                                emit_transpose(s4 + 1, tp_next.pop(0))
                            ci += 1
                    for st in range(4):
                        chain_v(st)
                        while len(tp_next) > (11 - ci) * ET // 12:
                            emit_transpose(s4 + 1, tp_next.pop(0))
                        ci += 1

            # ---------------- Phase B: attention ----------------
            with (
                tc.tile_pool(name="wop", bufs=1) as wop,
                tc.tile_pool(name="hidp", bufs=1) as hidp,
                tc.tile_pool(name="bop", bufs=1) as bop,
                tc.tile_pool(name="pp", bufs=6) as ppool,
                tc.tile_pool(name="stgB", bufs=3) as stgB,
                tc.tile_pool(name="aop", bufs=3) as aop,
                tc.tile_pool(name="ldp", bufs=4) as ldp,
                tc.tile_pool(name="blt", bufs=2) as blt,
            ):
                hid = [hidp.tile([128, RQ], BF16, name=f"hid{k}")
                       for k in range(4 * HL)]
                bo_bc = bop.tile([128, E], F32, name="bo_bc")
                bo_row = bop.tile([1, E], F32R, name="bo_row")
                nc.sync.dma_start(bo_row[:],
                                  _r(bo_d.rearrange("(o q) -> o q", o=1)))
                for ct in range(4):
                    ps_bo = psum.tile([128, 512], F32, tag="bc", bufs=1,
                                      name="ps_bo")
                    nc.tensor.matmul(
                        ps_bo[:], ones_row_r[:],
                        bo_row[:, ct * 512:(ct + 1) * 512],
                        start=True, stop=True)
                    nc.scalar.copy(bo_bc[:, ct * 512:(ct + 1) * 512],
                                   ps_bo[:])
                # Wo half-0 prefetch (cols 0:1024), used by phase C.
                wo0 = []
                for k in range(4 * HL):
                    hl_, src_ = k // 4, k % 4
                    eg = head_of(src_, hl_) * 128
                    t = wop.tile([128, 1024], BF16, tag="wo", name="wok",
                                 bufs=16)
                    nc.sync.dma_start(t[:], wo_d[eg:eg + 128, 0:1024])
                    wo0.append(t)

                pending_cc = [None]

                def emit_cc(hl):
                    nc.gpsimd.collective_compute(
                        "AllToAll", AL.bypass,
                        replica_groups=[list(range(N_CORES))],
                        ins=[a2a_in[hl].opt()],
                        outs=[a2a_out[hl].opt()])
                    for src_ in range(4):
                        k = hl * 4 + src_
                        la = ldp.tile([128, RQ], BF16, tag="la", name="la")
                        nc.sync.dma_start(
                            la[:],
                            a2a_out[hl][src_ * 128:(src_ + 1) * 128, :])
                        lb = ldp.tile([128, RQ], BF16, tag="lb", name="lb")
                        nc.sync.dma_start(
                            lb[:],
                            a2a_out[hl][(src_ + 4) * 128:(src_ + 5) * 128, :])
                        tmp = blt.tile([128, RQ], BF16, tag="tmp", name="tmp")
                        nc.vector.tensor_scalar(
                            tmp[:], lb[:], zsel[:, 1:2], None, AL.mult)
                        nc.vector.scalar_tensor_tensor(
                            hid[k][:], la[:], zsel[:, 0:1], tmp[:],
                            AL.mult, AL.add)

                for hl in range(HL):
                    for im in range(S4):
                        njt = 4 * im + 4
                        kept = [jt for jt in range(njt)
                                if jt >= 4 * im - B_SLOT[hl]]
                        first, last = kept[0], kept[-1]
                        ps_o = psum.tile([128, 512], F32, tag="o", bufs=2,
                                         name="ps_o")
                        ps_d = psum.tile([1, 512], F32, tag="d", bufs=1,
                                         name="ps_d")

                        def consume(jt_, p_):
                            nc.tensor.matmul(ps_d[:], ones_col[:], p_[:],
                                             start=(jt_ == first),
                                             stop=(jt_ == last))
                            nc.tensor.matmul(
                                ps_o[:],
                                v_sb[jt_][:, hl * 128:(hl + 1) * 128], p_[:],
                                start=(jt_ == first), stop=(jt_ == last))

                        pipe = deque()
                        for ji, jt in enumerate(kept):
                            ps_s = psum.tile([128, 512], F32, tag="mm",
                                             bufs=4, name="ps_s")
                            nc.tensor.matmul(
                                ps_s[:],
                                kT_sb[hl][:, jt * 128:(jt + 1) * 128],
                                qT_sb[hl][:, im * 512:(im + 1) * 512],
                                start=True, stop=True)
                            r = jt - 4 * im
                            p = ppool.tile([128, 512], BF16, tag="p",
                                           name="p")
                            if hl < 3:
                                # factorized: exp(scale*s + sl*(j-anchor));
                                # the per-column exp(-sl*(i-anchor)) factor
                                # cancels in the normalization, so only the
                                # 0/1 causal pattern is applied on diagonal
                                # tiles.
                                idx = (hl * S4 + im) * ST + jt
                                jv2 = bjv2[:, idx:idx + 1]
                                nc.scalar.activation(p[:], ps_s[:], AF.Exp,
                                                     bias=jv2,
                                                     scale=SCALE)
                                if r >= 0:
                                    nc.vector.tensor_tensor(
                                        p[:], p[:],
                                        cau[:, r * 512:(r + 1) * 512],
                                        AL.mult)
                            else:
                                # largest slopes: pre-exp bias+mask add
                                # (DVE, PSUM) then plain exp.
                                idx = im * ST + jt
                                jv = bjv[:, idx:idx + 1]
                                if r >= 0:
                                    in1 = bmask[:, r * 512:(r + 1) * 512]
                                else:
                                    in1 = bim[:, :512]
                                nc.vector.scalar_tensor_tensor(
                                    ps_s[:], ps_s[:], jv, in1,
                                    AL.add, AL.add)
                                nc.scalar.activation(p[:], ps_s[:], AF.Exp,
                                                     scale=SCALE)
                            pipe.append((jt, p))
                            if len(pipe) > 3:
                                consume(*pipe.popleft())
                        while pipe:
                            consume(*pipe.popleft())

                        sr1 = stgB.tile([1, 512], F32, tag="sd", name="sr1")
                        nc.vector.reciprocal_approx_fast(sr1[:], ps_d[:])
                        srb = stgB.tile([128, 512], F32, tag="sr",
                                        name="srb")
                        nc.gpsimd.partition_broadcast(srb[:], sr1[:])
                        ao = aop.tile([128, 512], BF16, tag="ao", name="ao")
                        nc.vector.scalar_tensor_tensor(
                            ao[:], ps_o[:], 0.0, srb[:], AL.bypass, AL.mult)
                        for dup in (0, 4):
                            nc.sync.dma_start(
                                a2a_in[hl][(im + dup) * 128:
                                           (im + dup + 1) * 128, :],
                                ao[:])
                        if im == 0 and pending_cc[0] is not None:
                            emit_cc(pending_cc[0])
                            pending_cc[0] = None
                    pending_cc[0] = hl
                emit_cc(pending_cc[0])

                # -------------- Phase C: output projection --------------
                with tc.tile_pool(name="stgC", bufs=4) as stgC:
                    acc_spec = [("mm", 4), ("mm", 4), ("mm", 4), ("mm", 4),
                                ("o", 2), ("o", 2), ("d", 1), ("bc", 1)]
                    for half in range(2):
                        if half == 0:
                            wo_tiles = wo0
                        else:
                            wo_tiles = []
                            for k in range(4 * HL):
                                hl_, src_ = k // 4, k % 4
                                eg = head_of(src_, hl_) * 128
                                t = wop.tile([128, 1024], BF16, tag="wo",
                                             name="wok2", bufs=16)
                                nc.sync.dma_start(
                                    t[:], wo_d[eg:eg + 128, 1024:2048])
                                wo_tiles.append(t)
                        pos = [psum.tile([128, 512], F32, tag=tg, bufs=bf_,
                                         name="ps_c")
                               for tg, bf_ in acc_spec]
                        for k in range(4 * HL):
                            wt_ = wo_tiles[k]
                            for rt in range(RT):
                                for cth in range(2):
                                    nc.tensor.matmul(
                                        pos[rt * 2 + cth][:],
                                        hid[k][:, rt * 128:(rt + 1) * 128],
                                        wt_[:, cth * 512:(cth + 1) * 512],
                                        start=(k == 0), stop=(k == 4 * HL - 1))
                        for rt in range(RT):
                            for cth in range(2):
                                ct = half * 2 + cth
                                so = stgC.tile([128, 512], F32, tag="soC",
                                               name="soC")
                                nc.vector.scalar_tensor_tensor(
                                    so[:], pos[rt * 2 + cth][:], 0.0,
                                    bo_bc[:, ct * 512:(ct + 1) * 512],
                                    AL.bypass, AL.add)
                                nc.sync.dma_start(
                                    out_d[rt * 128:(rt + 1) * 128,
                                          ct * 512:(ct + 1) * 512], so[:])

    nc.compile()
    return nc


def make_in_maps(x, Wqkv, bqkv, Wo, bo, seq=SEQ):
    import ml_dtypes
    x = np.asarray(x, np.float32)
    Wqkv = np.asarray(Wqkv, np.float32)
    bqkv = np.asarray(bqkv, np.float32)
    Wo = np.ascontiguousarray(
        np.asarray(Wo, np.float32).astype(ml_dtypes.bfloat16))
    bo = np.asarray(bo, np.float32)
    E = HIDDEN
    slopes = _slopes()
    jp = np.arange(128, dtype=np.float32)
    iif = np.arange(512, dtype=np.float32)
    bf16 = ml_dtypes.bfloat16
    in_maps = []
    for c in range(N_CORES):
        b, g = c // 4, c % 4
        heads = [head_of(g, hl) for hl in range(HL)]
        hcols = np.concatenate(
            [np.arange(h * HEAD, (h + 1) * HEAD) for h in heads])
        bjv = np.zeros((128, S4 * ST), np.float32)
        bim = np.zeros((128, 512), np.float32)
        bmask = np.zeros((128, 4 * 512), np.float32)
        bjv2 = np.zeros((128, 3 * S4 * ST), np.float32)
        cau = np.zeros((128, 4 * 512), np.float32)
        for r in range(4):
            cau[:, r * 512:(r + 1) * 512] = (
                iif[None, :] >= (128 * r + jp[:, None])).astype(np.float32)
        for hl in range(HL):
            sl_pre = slopes[heads[hl]] / SCALE   # pre-scale units
            sl = slopes[heads[hl]]               # post-scale units
            if hl < 3:
                # factorized path: column factor dropped (cancels in the
                # softmax normalization). Slot 2's larger slopes need a
                # mid-block anchor to keep the exponent in fp32 range.
                anchor = 256 if hl == 2 else 0
                for im in range(S4):
                    for jt in range(ST):
                        bjv2[:, (hl * S4 + im) * ST + jt] = sl * (
                            jp + 128 * jt - 512 * im - anchor)
            else:
                for im in range(S4):
                    for jt in range(ST):
                        bjv[:, im * ST + jt] = sl_pre * (
                            jt * 128 + jp - im * 512)
                bim[:, :] = -sl_pre * iif[None, :]
                for r in range(4):
                    blk = bmask[:, r * 512:(r + 1) * 512]
                    blk[:] = -sl_pre * iif[None, :]
                    keep = iif[None, :] >= (128 * r + jp[:, None])
                    blk[~keep] = NEG
        zsel = np.zeros((128, 2), np.float32)
        zsel[:, 0] = 1.0 if b == 0 else 0.0
        zsel[:, 1] = 1.0 - zsel[:, 0]
        castw = lambda a: np.ascontiguousarray(a.astype(bf16))
        in_maps.append({
            "x": np.ascontiguousarray(x[b, :seq]),
            "wq": castw(Wqkv[:, hcols]),
            "wk": castw(Wqkv[:, E + hcols]),
            "wv": castw(Wqkv[:, 2 * E + hcols]),
            "bq": np.ascontiguousarray(bqkv[hcols]),
            "bk": np.ascontiguousarray(bqkv[E + hcols]),
            "bv": np.ascontiguousarray(bqkv[2 * E + hcols]),
            "wo": Wo,
            "bo": bo.copy(),
            "bjv": bjv,
            "bim": np.ascontiguousarray(bim.astype(bf16)),
            "bmask": np.ascontiguousarray(bmask.astype(bf16)),
            "bjv2": bjv2,
            "cau": np.ascontiguousarray(cau.astype(bf16)),
            "zsel": zsel,
        })
    return in_maps


def unshard(outs, seq=SEQ):
    full = np.zeros((BATCH, seq, HIDDEN), np.float32)
    q = seq // 4
    for c in range(N_CORES):
        b, g = c // 4, c % 4
        full[b, g * q:(g + 1) * q, :] = outs[c]["out"]
    return full


_NC_CACHE = {}


def kernel(x, Wqkv, bqkv, Wo, bo):
    key = ("full", SEQ)
    if key not in _NC_CACHE:
        _NC_CACHE[key] = build_nc(SEQ)
    nc = _NC_CACHE[key]
    in_maps = make_in_maps(x, Wqkv, bqkv, Wo, bo)
    res = run_bass_kernel_spmd(nc, in_maps, core_ids=list(range(N_CORES)))
    return unshard(res.results)
